# revision 11
# baseline (speedup 1.0000x reference)
"""Bass/Trainium2 kernel for nn_CapLayer (dynamic-routing capsule layer), v5.

Same algebraic identity as v1 (see kernel.py docstring): with zero-init
routing logits the softmax stays uniform forever, so
    v[b, o, :] = squash((1/64) * sum_n pred[b, n, :])  for every o.

Changes vs the v1 baseline:
  - the spatial k-reduction is done on the PE via PSUM accumulation:
    u2[:, b, 4j:4j+4] = sum_k matmul(xt[:, b, 32k:32k+32], sel).  This
    removes every DVE TensorReduce (v1 spent ~18us of DVE time tracking the
    stream, and its pipeline lag added ~500ns to the tail).
  - wt shipped as fp16 (halves the weight DMA on the serialized DMA device)
    and consumed directly by the B-stage matmuls; u2 staged to SBUF as fp16
    so the B matmuls run at 1 PE cycle/row instead of 4.  No wt staging
    copy through DVE.
  - three unused const-AP memsets are elided from the Bass prologue,
    releasing the engine start barrier ~250ns earlier.
  - batch 7 accumulates in its own PSUM tile, so staging batches 0..6 to
    SBUF mid-stream cannot WAR-stall the final a2 ladder; only a [32,1,4]
    sliver remains between the last chunk and the B matmuls.
  - squash epilogue: ACT does square+row-sum then sqrt back-to-back (no
    cross-engine hop); DVE computes 1/(1+n2) in parallel and finishes with one
    fused (sbar*r)*rd tensor_scalar op.
"""

import contextlib
import json

import numpy as np

import concourse.bass as bass
import concourse.tile as tile
from concourse import mybir
from concourse.bass_utils import run_bass_kernel_spmd

N_CORES = 8
BS = 64
BPC = BS // N_CORES  # 8 batches per core
BM = BPC - 1         # batches staged by the early (main) pipeline
NCH = 1024           # num_shared * in_dim channels
HW = 256             # 16*16 spatial
NS = 32              # num shared groups
IN_DIM = 32
OUT_DIM = 64
NUM_OUT = 64
F32 = mybir.dt.float32
F16 = mybir.dt.float16

# stash of the last run's BassKernelResults for test harnesses
LAST_RESULTS = None
_NC_CACHE = None


@contextlib.contextmanager
def _skip_unused_const_memsets():
    """Bass.__init__ materializes four scalar const APs (0.0/1.0 f32,
    1.0 bf16, 127 u8) with serial Pool-engine memsets inside the engine
    start barrier.  This kernel only ever reads the f32 0.0 one (activation
    bias); eliding the other three releases the barrier ~250ns earlier."""
    holders = [
        cls for cls in (bass.BassGpSimd, *bass.BassGpSimd.__mro__,
                        bass.BassSharedVectorInterface)
        if "memset" in vars(cls)
    ]
    origs = {cls: vars(cls)["memset"] for cls in holders}

    def make_patched(orig):
        def patched(self, ap, constant):
            t = getattr(ap, "tensor", None)
            name = getattr(t, "name", "")
            if name in (
                "const-float32-1.0", "const-bfloat16-1.0", "const-uint8-127",
            ):
                return None
            if name == "const-float32-0.0" and self is not self.bass.vector:
                # keep the used const, but emit its memset on DVE — Pool's
                # prologue is the longest and gates the start barrier
                return orig(self.bass.vector, ap, constant)
            return orig(self, ap, constant)
        return patched

    for cls, orig in origs.items():
        cls.memset = make_patched(orig)
    try:
        yield
    finally:
        for cls, orig in origs.items():
            cls.memset = orig


def _split_multi_waits(bir: bytes) -> bytes:
    """The walrus build in this toolchain only accepts a single sync-wait
    command per instruction; Tile freely attaches several.  Rewrite the BIR so
    any instruction with N>1 waits is preceded by N-1 single-wait NoOps on the
    same engine — semantically identical, acceptable to this codegen."""
    j = json.loads(bir)
    ctr = [0]

    def fix_block(b):
        new = []
        for inst in b.get("instructions", []):
            si = inst.get("sync_info")
            if si:
                waits = si.get("on_wait") or []
                if len(waits) > 1:
                    for w in waits[:-1]:
                        ctr[0] += 1
                        new.append({
                            "debug": inst.get("debug", 0),
                            "engine": inst["engine"],
                            "ins": [],
                            "name": f"W-{ctr[0]}",
                            "opcode": "NoOp",
                            "outs": [],
                            "sync_info": {"on_update": [], "on_wait": [w]},
                        })
                    si["on_wait"] = [waits[-1]]
            new.append(inst)
        b["instructions"] = new
        for sb in b.get("blocks", []):
            fix_block(sb)

    for f in j.get("functions", []):
        for b in f.get("blocks", []):
            fix_block(b)
    return json.dumps(j).encode()


def _lean_teardown(nc):
    """Tile's epilogue is: completion drain -> all-engine barrier -> sem
    range-clear -> second barrier, ~530ns of which sit after the final
    store's completion semaphore.  The range-clear only exists so a
    re-execution of the NEFF starts with clean semaphores; the barrier sems
    themselves are outside the cleared range and the start-barrier protocol
    is self-cleaning (gather/release return to 0 every run).  So: clear at
    the START instead (inside the existing start barrier, before any sem in
    the range is touched) and delete everything after the completion drain.
    The program then ends the moment the store's semaphore lands."""
    fn = nc.m.functions[0]
    if len(fn.blocks) < 3:
        return
    b0, b2 = fn.blocks[0], fn.blocks[-1]
    l2 = b2.instructions
    insts2 = list(l2)
    if not insts2:
        return
    drain = insts2[0]
    si = drain.sync_info
    if type(drain).__name__ != "InstDrain" \
            or drain.engine != mybir.EngineType.SP \
            or si is None or len(si.on_wait) < 5:
        return  # unexpected epilogue shape: leave it alone
    clears = [i for i in insts2 if type(i).__name__ == "InstISA"]
    if len(clears) != 1:
        return
    clear = clears[0]
    # the clear must not wipe the barrier sems the start barrier relies on
    rng = clear.ant_dict
    first, last = rng["range_first"], rng["range_last"]
    for inst in list(b0.instructions):
        s = inst.sync_info
        if s is None:
            continue
        for ref in (*s.on_wait, *s.on_update):
            if "barrier" in (ref.ant_name or "") and first <= ref.id <= last:
                return  # clear would wipe a start-barrier sem: bail out
    # insert the clear at the head of Pool's block-0 stream (all user code
    # is gated behind the start barrier, which Pool arrives at afterwards)
    l0 = b0.instructions
    idx = None
    for i, ins in enumerate(l0):
        if ins.engine == mybir.EngineType.Pool \
                and type(ins).__name__ in ("InstDrain", "InstEventSemaphore"):
            idx = i
            break
    if idx is None:
        return
    for inst in insts2[1:]:
        l2.remove(inst)
    l0.insert(idx, clear)


def _strip_dead_register_moves(nc):
    """Bass's prologue seeds per-engine zero/broadcast registers with ~26
    RegisterMoves (50-96ns each, serial per engine) inside the start
    barrier.  Nothing in this kernel reads any of them — every regref
    appears exactly once, at its own move — so drop them and release the
    barrier earlier.  Verified by counting regref occurrences; bail if any
    register has a second reference."""
    import re as _re
    blob = nc.to_json_bytes().decode()
    counts = {}
    for reg in _re.findall(r'"regref": "([^"]+)"', blob):
        counts[reg] = counts.get(reg, 0) + 1
    fn = nc.m.functions[0]
    b0 = fn.blocks[0]
    l0 = b0.instructions
    for inst in list(l0):
        if type(inst).__name__ != "InstRegisterMove":
            continue
        outs = inst.outs
        regs = [getattr(o, "regref", None) for o in outs]
        if all(r is not None and counts.get(r, 0) <= 1 for r in regs):
            l0.remove(inst)


def _build(probe: str = ""):
    lvl = {"dma": 1, "a2": 3, "b": 4}.get(probe, 5)

    with _skip_unused_const_memsets():
        nc = bass.Bass()
    x = nc.dram_tensor("x", [BPC, NCH, HW], F32, kind="ExternalInput")
    wt = nc.dram_tensor("wt", [IN_DIM, NS, OUT_DIM], F16, kind="ExternalInput")
    # the out-caps axis of v is mathematically degenerate (identical for all
    # o) — the device emits only the unique [b, d] rows; the host unshard
    # step broadcasts to the full [b, o, d] shape.
    out = nc.dram_tensor("out", [BPC, OUT_DIM], F32, kind="ExternalOutput")

    with tile.TileContext(nc) as tc:
        with (
            tc.tile_pool(name="consts", bufs=1) as consts,
            tc.tile_pool(name="xp", bufs=17) as xp,
            tc.tile_pool(name="ep", bufs=1) as ep,
            tc.tile_pool(name="pp", bufs=1, space="PSUM") as pp,
        ):
            # weights straight from DRAM (fp16), no staging copy.  Issued on
            # the otherwise-idle Activation HWDGE queue so Pool stays empty
            # (a Pool-queue DMA would add SWDGE-ring setup to the prologue).
            wt_sb = consts.tile([IN_DIM, NS, OUT_DIM], F16)
            nc.scalar.dma_start(out=wt_sb, in_=wt[:])

            # group-selector matrix sel[c, g] = (c // 32 == g), f32 to match
            # the f32 xt lhsT in the A2 matmuls.
            sel_sb = consts.tile([128, 4], F32)
            nc.vector.memset(sel_sb, 0.0)
            for g in range(4):
                nc.vector.memset(sel_sb[32 * g:32 * (g + 1), g:g + 1], 1.0)

            # u2*[i, b, s] = usum[b, s, i], accumulated straight out of PE.
            # Batch 7 gets its own PSUM tile so the early staging copies
            # never WAR-stall its late a2 ladder; both stage into ONE SBUF
            # tile so the B matmuls see a contiguous [32, 8] lhsT.
            u2m = pp.tile([IN_DIM, BM, NS], F32)
            u2l = pp.tile([IN_DIM, 1, NS], F32)
            u2_sb = ep.tile([IN_DIM, BPC, NS], F16)
            sbar_ps = pp.tile([BPC, OUT_DIM], F32)

            # xv[p, j, b, m] = x[b, j*128 + p, m]
            xv = x.rearrange("b (j p) m -> p j b m", p=128)

            # Chunks = (channel block j, batch range [b0, b1)).  The A2
            # ladder consumes each chunk straight out of SBUF on the PE, so
            # chunk size only affects overlap granularity; shrink toward the
            # end to minimize the post-stream tail.
            chunks = []
            for j in range(6):
                chunks += [(j, 0, 4), (j, 4, 8)]
            chunks += [(6, 0, 2), (6, 2, 4), (6, 4, 6), (6, 6, 8)]
            chunks += [(7, 0, 2), (7, 2, 4), (7, 4, 6), (7, 6, 7), (7, 7, 8)]

            def a2_ladder(xt, j, b, brel):
                tgt = u2m[:, b, 4 * j:4 * j + 4] if b < BM \
                    else u2l[:, 0, 4 * j:4 * j + 4]
                for k in range(8):
                    # u2[i, b, 4j+g] += sum_p xt[p, b, 32k+i] * sel[p, g]
                    # — the spatial-k sum rides the PSUM accumulator.
                    nc.tensor.matmul(
                        out=tgt,
                        lhsT=xt[:, brel, 32 * k:32 * (k + 1)],
                        rhs=sel_sb[:],
                        start=(k == 0),
                        stop=(k == 7),
                        skip_group_check=True,
                    )

            def stage(j, main):
                if main:
                    nc.vector.tensor_copy(
                        out=u2_sb[:, 0:BM, 4 * j:4 * j + 4],
                        in_=u2m[:, :, 4 * j:4 * j + 4],
                    )
                else:
                    nc.vector.tensor_copy(
                        out=u2_sb[:, BM:BPC, 4 * j:4 * j + 4],
                        in_=u2l[:, :, 4 * j:4 * j + 4],
                    )

            def b_block(j):
                for g in range(4):
                    s = 4 * j + g
                    # sbar[b, o] += sum_i usum[b,s,i] * W[s,o,i] / 64
                    nc.tensor.matmul(
                        out=sbar_ps,
                        lhsT=u2_sb[:, :, s],
                        rhs=wt_sb[:, s, :],
                        start=(s == 0),
                        stop=(s == NS - 1),
                        skip_group_check=True,
                    )

            for (j, b0, b1) in chunks:
                nb = b1 - b0
                xt = xp.tile([128, nb, HW], F32, tag="xt", name=f"xt_{j}_{b0}")
                nc.sync.dma_start(out=xt, in_=xv[:, j, b0:b1, :])
                if lvl < 3:
                    continue
                for b in range(b0, b1):
                    a2_ladder(xt, j, b, b - b0)
                if lvl < 4:
                    continue
                # stage each batch group of block j as soon as its ladders
                # are emitted; fire the B matmuls once both halves landed
                if b1 == BPC:
                    if b0 < BM:
                        stage(j, main=True)
                    stage(j, main=False)
                    b_block(j)
                elif b1 == BM:
                    stage(j, main=True)

            if lvl < 5:
                dump = ep.tile([BPC, OUT_DIM], F32)
                nc.vector.memset(dump, 0.0)
                nc.sync.dma_start(out=out[:], in_=dump)
                orig_to_json_p = nc.to_json_bytes
                nc.to_json_bytes = lambda: _split_multi_waits(orig_to_json_p())
                return nc

            # squash on [8, 64]: v = sbar * sqrt(n2)/(1+n2), n2 = |sbar|^2.
            # (wt is pre-scaled by 1/64 on the host, so sbar_ps IS sbar.)
            # ACT fuses square+row-sum then sqrt (same engine, no hop); DVE
            # computes d=1+n2 in parallel and finishes with one fused
            # (sbar*r)/d op.
            sq = ep.tile([BPC, OUT_DIM], F32)
            n2 = ep.tile([BPC, 1], F32)
            nc.scalar.activation(
                out=sq, in_=sbar_ps,
                func=mybir.ActivationFunctionType.Square,
                accum_out=n2,
            )
            r = ep.tile([BPC, 1], F32)
            nc.scalar.sqrt(out=r, in_=n2)
            d = ep.tile([BPC, 1], F32)
            nc.vector.tensor_scalar_add(out=d, in0=n2, scalar1=1.0)
            rd = ep.tile([BPC, 1], F32)
            nc.vector.reciprocal(out=rd, in_=d)
            vrow = ep.tile([BPC, OUT_DIM], F32)
            nc.vector.tensor_scalar(
                out=vrow, in0=sbar_ps, scalar1=r, scalar2=rd,
                op0=mybir.AluOpType.mult, op1=mybir.AluOpType.mult,
            )
            nc.sync.dma_start(out=out[:], in_=vrow)

    # Both post-passes verify their preconditions and bail gracefully, but
    # guard against surprises anyway: losing one of them costs ~400ns,
    # while an exception here would cost the whole kernel.
    for _pass in (_lean_teardown, _strip_dead_register_moves):
        try:
            _pass(nc)
        except Exception:
            pass
    # every compile path (native walrus + bass2jax/axon) serializes via
    # to_json_bytes — splice the single-wait rewrite in there
    orig_to_json = nc.to_json_bytes
    nc.to_json_bytes = lambda: _split_multi_waits(orig_to_json())
    return nc


def kernel(x: np.ndarray, W: np.ndarray, trace: bool = False) -> np.ndarray:
    global LAST_RESULTS, _NC_CACHE
    x = np.ascontiguousarray(np.asarray(x, dtype=np.float32)).reshape(BS, NCH, HW)
    W = np.asarray(W, dtype=np.float32)

    # [i, s, o], pre-scaled so the PE B-stage directly produces sbar;
    # fp16 halves the weight DMA and runs the B matmuls at 1 cycle/row.
    wt = np.ascontiguousarray(W.transpose(2, 0, 1) * (1.0 / 64.0)).astype(np.float16)

    if _NC_CACHE is None:
        _NC_CACHE = _build()
    nc = _NC_CACHE
    in_maps = [
        {"x": np.ascontiguousarray(x[c * BPC:(c + 1) * BPC]), "wt": wt}
        for c in range(N_CORES)
    ]
    res = run_bass_kernel_spmd(nc, in_maps, core_ids=list(range(N_CORES)), trace=trace)
    LAST_RESULTS = res
    rows = np.concatenate([r["out"] for r in res.results], axis=0)  # [64, 64]
    # unshard: materialize the degenerate out-caps axis (v is identical for
    # every o — see the module docstring)
    return np.ascontiguousarray(
        np.broadcast_to(rows[:, None, :], (BS, NUM_OUT, OUT_DIM))
    )
